# revision 21
# baseline (speedup 1.0000x reference)
"""Trainium2 Bass kernel for MultiHeadAttention (RMSNorm + MHA + residual).

Reference computation (B=2, S=2048, D=1024, H=16):
    xn = x * rsqrt(mean(x^2, -1) + 1e-12) * gamma
    q/k/v = (xn @ W{q,k,v}.T) split into heads
    attn  = softmax(q k^T / sqrt(64)) v          (mask is zeros)
    out   = xn + (attn @ Wo.T)

Sharding: tensor-parallel over heads (2 heads/core on 8 cores) for
QKV/scores/softmax/attn-V, then an AllToAll switches to token sharding
(512 tokens/core) for the output projection + residual.

Normalization strategy (keeps the PE busy from t=0): projections run on
RAW feature-major x; rstd is folded in afterwards --
  * rstd is computed from each core's own 512-token f32 slice and
    AllGathered (2 KB), so no full-x square pass and no DMA broadcast.
  * Q: fused into the psum->SBUF evacuation multiply (needs rstdB, a
    partition-replicated [128, TOK] tile built by tiny PE matmuls
    against a ones vector).
  * K: left raw; exp(score * rstd_k * rstd_q / 8) folds rstd_k via the
    per-partition `scale` AP of the activation (partition = key token).
  * V: computed directly token-major (out[tok, feat] accumulation), so
    rstd_v is a per-partition tensor_scalar on evacuation; the softmax
    denominator comes from 64 ones-columns appended to V (Z lands on
    psum partitions 64..127 of the attn@V matmul).

Engine budget per core: PE ~400k col-cycles, ACT does ONLY exp (+local
squares), DVE does recip/Z-divide/Q-evac, gpsimd does K/V evacuations.
PSUM: tag "ps" [128,1024]f32 x2 (4 banks) + tag "pa" [128,512]f32 x4
(4 banks) = all 8 banks, tags shared across phases.
"""

import numpy as np
import ml_dtypes

import concourse.bacc as bacc
import concourse.mybir as mybir
import concourse.tile as tile
from concourse.bass_utils import run_bass_kernel_spmd

F32 = mybir.dt.float32
BF16 = mybir.dt.bfloat16
AF = mybir.ActivationFunctionType

NCORES = 8
D = 1024
H = 16
DH = 64            # head dim
HPC = H // NCORES  # heads per core
FPC = HPC * DH     # attn features per core


def build(B=2, S=2048):
    TOK = B * S
    NT = TOK // 128      # token tiles (32)
    IC = D // 128        # input-feature chunks (8)
    TPC = TOK // NCORES  # tokens per core (512)
    LT = TPC // 128      # local token tiles (4)
    KT = S // 128        # key tiles per batch (16)
    QCH = TPC            # q-block size (one A2A shard, 512)
    QQ = S // QCH        # q-blocks per batch (4)
    assert TPC % 128 == 0 and S % TPC == 0 and TPC <= 512

    nc = bacc.Bacc("TRN2", target_bir_lowering=False, debug=False,
                   num_devices=NCORES)
    xt_d = nc.dram_tensor("xt", [D, TOK], BF16, kind="ExternalInput")
    xres_d = nc.dram_tensor("xres", [TPC, D], F32, kind="ExternalInput")
    wq_d = nc.dram_tensor("wq", [D, FPC], BF16, kind="ExternalInput")
    wk_d = nc.dram_tensor("wk", [D, FPC], BF16, kind="ExternalInput")
    wv_d = nc.dram_tensor("wv", [D, FPC], BF16, kind="ExternalInput")
    wo_d = nc.dram_tensor("wo", [D, D], BF16, kind="ExternalInput")
    gamma_d = nc.dram_tensor("gamma", [1, D], F32, kind="ExternalInput")
    out_d = nc.dram_tensor("out", [TPC, D], F32, kind="ExternalOutput")
    DBG = bool(__import__("os").environ.get("K_DBG"))
    if DBG:
        dbg1_d = nc.dram_tensor("dbg1", [128, TOK], BF16,
                                kind="ExternalOutput")
        dbg2_d = nc.dram_tensor("dbg2", [128, HPC * 128], BF16,
                                kind="ExternalOutput")
        dbg3_d = nc.dram_tensor("dbg3", [128, TOK], BF16,
                                kind="ExternalOutput")

    with tile.TileContext(nc) as tc:
        with (
            tc.tile_pool(name="sb", bufs=1) as sb,
            tc.tile_pool(name="ps", bufs=1, space="PSUM") as ps,
            tc.tile_pool(name="dram", bufs=1, space="DRAM") as dpool,
        ):
            rg_in = dpool.tile([128, LT], F32)
            rg_out = dpool.tile([NCORES, 128, LT], F32)
            rg2 = dpool.tile([32, 128], BF16)
            bounce_in = dpool.tile([NCORES, FPC, TPC], BF16)
            bounce_out = dpool.tile([NCORES, FPC, TPC], BF16)

            # ---- input DMAs (ordered by first use) ----
            wq_sb = sb.tile([128, IC, FPC], BF16, tag="wq")
            wk_sb = sb.tile([128, IC, FPC], BF16, tag="wk")
            wv_sb = sb.tile([128, IC, FPC], BF16, tag="wv")
            for w_sb, w_d in ((wk_sb, wk_d), (wv_sb, wv_d), (wq_sb, wq_d)):
                nc.sync.dma_start(
                    w_sb[:], w_d[:].rearrange("(ic p) f -> p ic f", p=128))
            xres_sb = sb.tile([128, LT, D], F32, tag="xres")
            for lt in range(LT):
                nc.sync.dma_start(xres_sb[:, lt, :],
                                  xres_d[lt * 128:(lt + 1) * 128, :])
            xt = [sb.tile([128, TOK], BF16, tag=f"xt{ic}", name=f"xt{ic}")
                  for ic in range(IC)]
            for ic in range(IC):
                nc.sync.dma_start(xt[ic][:], xt_d[ic * 128:(ic + 1) * 128, :])
            gamma_sb = sb.tile([128, D], F32, tag="gamma")
            nc.sync.dma_start(gamma_sb[:], gamma_d[:].to_broadcast([128, D]))
            wo_sb = sb.tile([128, IC, D], BF16, tag="wo")
            nc.sync.dma_start(
                wo_sb[:], wo_d[:].rearrange("(ic p) f -> p ic f", p=128))

            # V staging: [128 tok, h, dh|ones]; 64 ones-cols per head for the
            # Z row trick (denominator lands on psum partitions 64..127)
            v_sb = [sb.tile([128, HPC, 128], BF16, tag=f"v{t}",
                            name=f"v{t}") for t in range(NT)]
            for t in range(NT):
                nc.vector.memset(v_sb[t][:, :, DH:128], 1.0)

            # ---- phase A: local rstd from own f32 slice, AllGather ----
            sms_loc = sb.tile([128, LT], F32, tag="smsl")
            for lt in range(LT):
                sq_t = sb.tile([128, D], BF16, tag="sq", bufs=2)
                ssq = sb.tile([128, 1], F32, tag="ssq", bufs=2)
                nc.scalar.activation(sq_t[:], xres_sb[:, lt, :], AF.Square,
                                     accum_out=ssq[:])
                nc.scalar.activation(sms_loc[:, lt:lt + 1], ssq[:], AF.Sqrt,
                                     scale=1.0 / D)
            rstd_loc = sb.tile([128, LT], F32, tag="rstdl")
            nc.vector.reciprocal(rstd_loc[:], sms_loc[:])
            nc.sync.dma_start(rg_in[:], rstd_loc[:])
            nc.gpsimd.collective_compute(
                "AllGather", mybir.AluOpType.bypass,
                replica_groups=[list(range(NCORES))],
                ins=[rg_in[:].opt()],
                outs=[rg_out[:].opt()])
            # rstd_all[p, c, lt]: token tile index = c*LT + lt
            rstd_all = sb.tile([128, NCORES, LT], F32, tag="rstda")
            nc.sync.dma_start(rstd_all[:], rg_out[:].rearrange("c p l -> p c l"))
            rstd_flat = rstd_all[:].rearrange("p c l -> p (c l)")
            # exp scale: rstd_k / 8 per key token (partition = token in tile)
            s8_all = sb.tile([128, NT], F32, tag="s8")
            nc.vector.tensor_scalar_mul(s8_all[:], rstd_flat, 0.125)
            rstd_bf = sb.tile([128, NT], BF16, tag="rstdbf")
            nc.vector.tensor_copy(rstd_bf[:], rstd_flat)
            # rstdT[t, p]: DVE 32x32 block transposes at matching offsets
            rstdT = sb.tile([32, 128], BF16, tag="rstdT")
            for b32 in range(4):
                nc.vector.transpose(rstdT[0:32, b32 * 32:(b32 + 1) * 32],
                                    rstd_bf[b32 * 32:(b32 + 1) * 32, 0:32])


            # residual base: xg = xres * rstd * gamma (off critical path)
            xg_sb = sb.tile([128, LT, D], F32, tag="xg")
            for lt in range(LT):
                nc.vector.tensor_scalar_mul(xg_sb[:, lt, :], xres_sb[:, lt, :],
                                            rstd_loc[:, lt:lt + 1])
                nc.vector.tensor_mul(xg_sb[:, lt, :], xg_sb[:, lt, :],
                                     gamma_sb[:])

            # ---- phase B1: K projection (raw), gpsimd evacuation ----
            KTt = sb.tile([128, TOK], BF16, tag="kt")
            for pr in range(4):           # pairs of 512-token groups
                pk = ps.tile([128, 1024], F32, tag="ps", bufs=2, name=f"pk{pr}")
                for ic in range(IC):
                    for g in range(2):
                        g0 = (pr * 2 + g) * 512
                        nc.tensor.matmul(
                            pk[:, g * 512:(g + 1) * 512], wk_sb[:, ic, :],
                            xt[ic][:, g0:g0 + 512],
                            start=(ic == 0), stop=(ic == IC - 1))
                nc.scalar.copy(KTt[:, pr * 1024:(pr + 1) * 1024], pk[:])

            # ---- phase B2: V projection, direct token-major + rstd fold ----
            # one accumulation group per psum BANK (psum accumulate
            # granularity is the 2KB bank -- interleaved slice-groups within
            # a bank corrupt each other)
            for t in range(NT):
                pv = ps.tile([128, 512], F32, tag="pa", bufs=4, name=f"pv{t}")
                for ic in range(IC):
                    nc.tensor.matmul(
                        pv[:, 0:128],
                        xt[ic][:, t * 128:(t + 1) * 128], wv_sb[:, ic, :],
                        start=(ic == 0), stop=(ic == IC - 1))
                nc.vector.tensor_scalar_mul(
                    v_sb[t][:, :, 0:DH],
                    pv[:, 0:128].rearrange("p (h f) -> p h f", h=HPC),
                    rstd_flat[:, t:t + 1])

            # ---- phase B3: rstd broadcast [128, TOK] via DRAM bounce ----
            # (256B descriptors: each tile re-reads one 128-col rstdT row)
            nc.sync.dma_start(rg2[:], rstdT[:])
            rstdB = sb.tile([128, TOK], BF16, tag="rstdB")
            for tt in range(NT):
                nc.sync.dma_start(
                    rstdB[:, tt * 128:(tt + 1) * 128],
                    rg2[tt:tt + 1, :].to_broadcast([128, 128]))

            # ---- phase B4: Q projection (raw) + fused rstd_q/norm evac ----
            QT = sb.tile([128, TOK], BF16, tag="qt")
            for pr in range(4):
                pq = ps.tile([128, 1024], F32, tag="ps", bufs=2, name=f"pq{pr}")
                for ic in range(IC):
                    for g in range(2):
                        g0 = (pr * 2 + g) * 512
                        nc.tensor.matmul(
                            pq[:, g * 512:(g + 1) * 512], wq_sb[:, ic, :],
                            xt[ic][:, g0:g0 + 512],
                            start=(ic == 0), stop=(ic == IC - 1))
                nc.vector.tensor_mul(QT[:, pr * 1024:(pr + 1) * 1024], pq[:],
                                     rstdB[:, pr * 1024:(pr + 1) * 1024])

            if DBG:
                nc.sync.dma_start(dbg1_d[:], QT[:])
                nc.sync.dma_start(
                    dbg2_d[:], v_sb[5][:].rearrange("p h f -> p (h f)"))
                nc.sync.dma_start(dbg3_d[:], KTt[:])

            # ---- phase C: attention (transposed scores, fused Z) ----
            for b in range(B):
                for qq in range(QQ):
                    q0 = b * S + qq * QCH
                    dst = q0 // TPC
                    pa = [ps.tile([128, QCH], F32, tag="pa", bufs=4,
                                  name=f"pa{h}_{b}_{qq}")
                          for h in range(HPC)]
                    for kt in range(KT):
                        gt = b * KT + kt
                        k0 = b * S + kt * 128
                        p_s = ps.tile([128, HPC * QCH], F32, tag="ps", bufs=2,
                                      name=f"psc_{b}_{qq}_{kt}")
                        for h in range(HPC):
                            lo = h * DH
                            nc.tensor.matmul(
                                p_s[:, h * QCH:(h + 1) * QCH],
                                KTt[lo:lo + DH, k0:k0 + 128],
                                QT[lo:lo + DH, q0:q0 + QCH],
                                start=True, stop=True)
                        e_t = sb.tile([128, HPC * QCH], BF16, tag="e", bufs=3)
                        nc.scalar.activation(e_t[:], p_s[:], AF.Exp,
                                             scale=s8_all[:, gt:gt + 1])
                        for h in range(HPC):
                            nc.tensor.matmul(
                                pa[h][:], v_sb[gt][:, h, :],
                                e_t[:, h * QCH:(h + 1) * QCH],
                                start=(kt == 0), stop=(kt == KT - 1))
                    for h in range(HPC):
                        rz = sb.tile([64, QCH], F32, tag="rz", bufs=2)
                        nc.vector.reciprocal(rz[:], pa[h][64:128, :])
                        an = sb.tile([64, QCH], BF16, tag="an", bufs=2)
                        nc.vector.tensor_mul(an[:], pa[h][0:64, :], rz[:])
                        nc.sync.dma_start(
                            bounce_in[dst, h * DH:(h + 1) * DH, :], an[:])

            # ---- phase D: all-to-all (head-shard -> token-shard) ----
            nc.gpsimd.collective_compute(
                "AllToAll", mybir.AluOpType.bypass,
                replica_groups=[list(range(NCORES))],
                ins=[bounce_in[:].opt()],
                outs=[bounce_out[:].opt()])

            # ---- phase E: output projection + residual, token-sharded ----
            at_all = sb.tile([128, NCORES, TPC], BF16, tag="at")
            nc.sync.dma_start(at_all[:],
                              bounce_out[:].rearrange("s f t -> f s t"))
            for lt in range(LT):
                t0 = lt * 128
                po = ps.tile([128, 1024], F32, tag="ps", bufs=2,
                             name=f"po{lt}")
                for ng in range(2):
                    for ic in range(IC):
                        nc.tensor.matmul(
                            po[:, ng * 512:(ng + 1) * 512],
                            at_all[:, ic, t0:t0 + 128],
                            wo_sb[:, ic, ng * 512:(ng + 1) * 512],
                            start=(ic == 0), stop=(ic == IC - 1))
                ot = sb.tile([128, D], F32, tag="ot", bufs=2)
                nc.vector.tensor_add(ot[:], xg_sb[:, lt, :], po[:])
                nc.sync.dma_start(out_d[t0:t0 + 128, :], ot[:])

    nc.compile()
    return nc


_CACHE = {}


def _get_nc(B=2, S=2048):
    key = (B, S)
    if key not in _CACHE:
        _CACHE[key] = build(B, S)
    return _CACHE[key]


def make_in_maps(x, Wq, Wk, Wv, Wo, gamma, B, S):
    TOK = B * S
    TPC = TOK // NCORES
    bf = ml_dtypes.bfloat16
    x2d = np.ascontiguousarray(np.asarray(x, np.float32).reshape(TOK, D))
    xt = np.ascontiguousarray(x2d.T.astype(bf))
    gam = np.asarray(gamma, np.float32).reshape(D)
    woT = np.ascontiguousarray(np.asarray(Wo, np.float32).T.astype(bf))
    gamma_in = np.ascontiguousarray(gam.reshape(1, D))
    in_maps = []
    for c in range(NCORES):
        fs = slice(c * FPC, (c + 1) * FPC)
        m = {
            "xt": xt,
            "xres": np.ascontiguousarray(x2d[c * TPC:(c + 1) * TPC]),
            "wo": woT,
            "gamma": gamma_in,
        }
        for name, W in (("wq", Wq), ("wk", Wk), ("wv", Wv)):
            Wc = np.asarray(W, np.float32)[fs, :] * gam[None, :]
            m[name] = np.ascontiguousarray(Wc.T.astype(bf))
        in_maps.append(m)
    return in_maps


def kernel(x, attn_mask, Wq, Wk, Wv, Wo, gamma, _trace=False):
    B, S, _ = np.asarray(x).shape
    nc = _get_nc(B, S)
    in_maps = make_in_maps(x, Wq, Wk, Wv, Wo, gamma, B, S)
    res = run_bass_kernel_spmd(nc, in_maps, core_ids=list(range(NCORES)),
                               trace=_trace)
    out = np.concatenate([res.results[c]["out"] for c in range(NCORES)], axis=0)
    out = out.reshape(B, S, D).astype(np.float32)
    if _trace:
        kernel.last_results = res
    return out
